# revision 28
# baseline (speedup 1.0000x reference)
"""Trainium2 Bass kernel for nn_MultiHeadSelfAttention_62646392979761.

Math (per the buggy-einsum reference): per position s, heads attend to heads:
  Q,K,V = x@W{q,k,v}.T + b  (N,S,H,D);  scores[s] = Q[s]K[s]^T/8 (16x16);
  A = softmax_j;  AV[s] = A[s]V[s];  out2 = scrambled flat reshape;
  final = out2@Wo.T + bo.

Sharding: 8 cores x 2048 rows of the flattened (16384, 1024) x. Attention is
position-local; the scramble groups 16 consecutive positions, which never
cross a 2048-row shard. Zero cross-core communication.

Per-core pipeline (positions in 4 groups of 512, each 4 subtiles of 128).
Indices: position s = 128*sub + 16*G + w, pair-half p = w%2, c' = w//2%8,
pair g = 8*G + c' (u = g%4, q = g//4), head i = 2c+a.
  1. QT/KT projections transposed (stationary = weight chunk, moving = xT):
     psum [128 f=(a,d), 512 s] per f-chunk; DVE bias evac -> qt/kt [(a,d),(c,s)].
  2. V projection natural (stationary = xT chunk): vn [128 s, 1024 (j,d)],
     bounced to DRAM for the VBLK4 gather.
  3. QSTK [(p,d), 16g+i] / KBLK [(p,d), 32g+16p+j block-diag] via 4 strided
     DVE copies each from qt/kt. VBLK4 [(u,p,j), 128q+64p+d block-diag in p]
     via 8 gather DMAs (u,p) from the DRAM bounce: one [128,128] stationary
     slab holds the V of 4 pairs (8 positions). Structural zeros memset per
     rep on GPSIMD.
  4. Scores^T: 16 pair-group matmuls (k=128, m=32 col-quadrant-rotated, n=64,
     stationary KBLK slab, moving QSTK slice) -> sco psum [(u,p,j), (q,i)],
     one [128,1024] bank-pair per subtile.
  5. Softmax: 4 ACT exps (one per u band) -> E bf16; Z = mask-matmul (sums j
     over partitions, padded to m=32); DVE reciprocal into a memset-once
     const ring (32-row operands: 8-row matmul operands FAULT real HW);
     Zb = maskT-matmul broadcast; A2BIG [(u,p,j), 64q+16u+i block-diag in u]
     = E*Zb via 4 DVE muls (fused normalize + diagonal placement).
  6. AV: 16 matmuls per subtile (k=128, m=128, n=64: stationary VBLK4 slab,
     moving A2BIG slice) -> avt psum [(p,d), 64q_loc+16u+i] x2 halves.
     (Baseline did 64 n=16 matmuls w/ [32,128] stationaries: LDWEIGHTS-bound.)
  7. One strided ACT copy per avt half -> out2T [128 (p,d), (c',sub,G,i)].
  Stages run as a 3-deep skewed pipeline: scores(n) | Z(n-1) | bcast(n-2) |
  AV(n-3), so the exp->Z->recip->bcast->mul chain of subtile n resolves a
  full subtile before any PE instruction that needs it issues (the PE queue
  is in-order; un-skewed, Z/bcast head-of-line-block the ready AV matmuls).
  Ring depths follow the skew: vblk4 x8 (a whole sg's gathers are emitted in
  the sg head, 4 ahead of their AV), a2big/zr/kblk x2, esb x4. zp keeps its
  own psum tag: pf shares the ppj ring instead (a same-tag ring is only safe
  when no allocation interleaves a live tile's alloc->last-read window).
  8. Final projection: stationary WoPT chunks, moving out2T -> psum
     [128 f, 512 s'], + bo (DVE) -> out_d (1024, 2048) f32.
Weights/bias/masks load on the Activation HWDGE queue (overlaps the rep
boundary: their WAR hazards clear mid-rep, so the reload for rep k+1 runs
under rep k's tail compute); xt loads split into 4 ce-chunks on the SP queue
so rep k+1's first QK matmuls start ~1us after rep k's stores issue.
Host: transposes x/weights, permutes Wo rows, post-scatters out columns
(col = 16G + i -> row i*256 + s0/16 + G).
Note: row-quadrant tile_position (32u, 0) on AV matmuls compiles but faults
real HW (NRT_EXEC_UNIT_UNRECOVERABLE) — everything here stays at (0,0).
"""

import math
import os
import numpy as np
import ml_dtypes

os.environ.setdefault("NEURON_RT_RESET_CORES", "1")

ROWS = 2048
NB, SB, EB, HB, DB = 4, 4096, 1024, 16, 64

_CACHE = {}


def _split_waits_json(bir_bytes):
    """This env's walrus accepts only ONE embedded sync-wait per TPB
    instruction (NEURON_ISA_TPB_EVENTS has a single wait slot) but Tile emits
    several. Split excess on_wait entries onto standalone EventSemaphore
    instructions inserted just before, on the same engine — semantically
    identical on in-order engine queues."""
    import json
    d = json.loads(bir_bytes)
    for fn in d.get('functions', []):
        for bb in (fn.get('basic_blocks') or fn.get('blocks') or []):
            out = []
            for inst in bb.get('instructions', []):
                si = inst.get('sync_info')
                w = (si or {}).get('on_wait') or []
                if len(w) > 1:
                    for k, extra in enumerate(w[:-1]):
                        out.append({
                            'debug': inst.get('debug', 0),
                            'engine': inst['engine'],
                            'ins': [], 'outs': [],
                            'name': f"{inst['name']}-sw{k}",
                            'opcode': 'EventSemaphore',
                            'sync_info': {'on_wait': [extra], 'on_update': []},
                        })
                    si['on_wait'] = [w[-1]]
                out.append(inst)
            bb['instructions'] = out
    return json.dumps(d).encode()


def _install_birpatch():
    import concourse.bass_utils as bu
    import concourse.bass2jax as b2j
    if getattr(bu.compile_bir_kernel, '_waitsplit', False):
        return
    orig = bu.compile_bir_kernel

    def patched(bir_json, tmpdir, neff_name="file.neff"):
        return orig(_split_waits_json(bir_json), tmpdir, neff_name)

    patched._waitsplit = True
    bu.compile_bir_kernel = patched
    b2j.compile_bir_kernel = patched


def _build_bass(reps=1):
    import concourse.bass as bass
    import concourse.tile as tile
    from concourse import mybir
    # timing ablations (never set in the graded path): noav, nosm, noscores,
    # nogather strip stages to attribute HW time per stage
    _ab = set(os.environ.get("KABLATE", "").split(","))

    bf16 = mybir.dt.bfloat16
    f32 = mybir.dt.float32
    AF = mybir.ActivationFunctionType

    nc = bass.Bass(trn_type="TRN2")
    xt_d = nc.declare_dram_parameter("xt", [1024, ROWS], bf16, isOutput=False)
    wqT_d = nc.declare_dram_parameter("wqt", [1024, 1024], bf16, isOutput=False)
    wkT_d = nc.declare_dram_parameter("wkt", [1024, 1024], bf16, isOutput=False)
    wvT_d = nc.declare_dram_parameter("wvt", [1024, 1024], bf16, isOutput=False)
    woT_d = nc.declare_dram_parameter("wot", [1024, 1024], bf16, isOutput=False)
    bias_d = nc.declare_dram_parameter("bias", [128, 1048], f32, isOutput=False)
    mask_d = nc.declare_dram_parameter("mask", [128, 32], bf16, isOutput=False)
    maskT_d = nc.declare_dram_parameter("maskt", [32, 128], bf16, isOutput=False)
    out_d = nc.declare_dram_parameter("out", [1024, ROWS], bf16, isOutput=True)

    from contextlib import ExitStack
    with ExitStack() as ctx:
        tc = ctx.enter_context(tile.TileContext(nc))
        const = ctx.enter_context(tc.tile_pool(name="const", bufs=1))
        work = ctx.enter_context(tc.tile_pool(name="work", bufs=2))
        sfx = ctx.enter_context(tc.tile_pool(name="sfx", bufs=4))
        vnp = ctx.enter_context(tc.tile_pool(name="vnp", bufs=3))
        ppj = ctx.enter_context(tc.tile_pool(name="ppj", bufs=2, space="PSUM"))
        psc = ctx.enter_context(tc.tile_pool(name="psc", bufs=2, space="PSUM"))
        pav = ctx.enter_context(tc.tile_pool(name="pav", bufs=2, space="PSUM"))
        psh = ctx.enter_context(tc.tile_pool(name="psh", bufs=2, space="PSUM"))
        drp = ctx.enter_context(tc.tile_pool(name="drp", bufs=4, space="DRAM"))

        if reps != 1:
            ctx.enter_context(tc.For_i(0, reps))

        # ---- persistent tensors (reloaded every rep for honest timing) ----
        wq_sb = const.tile([128, 8192], bf16, tag="wq")
        wk_sb = const.tile([128, 8192], bf16, tag="wk")
        wv_sb = const.tile([128, 8192], bf16, tag="wv")
        wo_sb = const.tile([128, 8192], bf16, tag="wo")
        bias_sb = const.tile([128, 24], f32, tag="bias")
        biasv_sb = const.tile([128, 1024], f32, tag="biasv")
        mask_sb = const.tile([128, 32], bf16, tag="mask")
        maskT_sb = const.tile([32, 128], bf16, tag="maskt")
        out2t = const.tile([128, 16384], bf16, tag="o2t", name="o2t")
        kblks = [const.tile([128, 2048], bf16, tag=f"kblk{b}", name=f"kblk{b}")
                 for b in range(2)]
        vblk4s = [const.tile([128, 2048], bf16, tag=f"vblk4{b}", name=f"vblk4{b}")
                  for b in range(8)]
        a2bigs = [const.tile([128, 1024], bf16, tag=f"a2big{b}", name=f"a2big{b}")
                  for b in range(2)]
        zrs = [const.tile([32, 256], bf16, tag=f"zr{b}", name=f"zr{b}")
               for b in range(2)]
        for b in range(2):
            nc.gpsimd.memset(zrs[b][:], 0.0)
        for b in range(2):
            nc.gpsimd.memset(kblks[b][:], 0.0)
            nc.gpsimd.memset(a2bigs[b][:], 0.0)
        for b in range(8):
            nc.gpsimd.memset(vblk4s[b][:], 0.0)

        # weights + bias/masks on the ACT HWDGE queue: frees the SP queue for
        # activations and lets the rep-(k+1) reload overlap rep k's tail.
        # Queue order matters in the For_i steady state (in-order queue, each
        # DMA also waits its WAR hazard): tiny qkbo-bias first (WAR clears at
        # rep end but loads in ~30ns), then wq/wk (WAR ~75% of prior rep),
        # v-bias, wv, wo, masks.
        nc.scalar.dma_start(bias_sb[:], bias_d[:, 0:24])
        # weight layout: sb[p, 1024*ce + 128*cf + m] = W_T[128*ce + p, 128*cf + m]
        for sb, d in ((wq_sb, wqT_d), (wk_sb, wkT_d)):
            nc.scalar.dma_start(
                sb[:].rearrange("p (ce f) -> p ce f", ce=8),
                d[:].rearrange("(ce p) f -> p ce f", ce=8),
            )
        nc.scalar.dma_start(biasv_sb[:], bias_d[:, 24:1048])
        for sb, d in ((wv_sb, wvT_d), (wo_sb, woT_d)):
            nc.scalar.dma_start(
                sb[:].rearrange("p (ce f) -> p ce f", ce=8),
                d[:].rearrange("(ce p) f -> p ce f", ce=8),
            )
        nc.scalar.dma_start(mask_sb[:], mask_d[:])
        nc.scalar.dma_start(maskT_sb[:], maskT_d[:])
        bqT = bias_sb[:, 0:8]
        bkT = bias_sb[:, 8:16]
        boT = bias_sb[:, 16:24]
        bvR = biasv_sb[:, 0:1024]

        def _emit_final(fg):
            if "nofinal" in _ab:
                return
            for cf in range(8):
                # pf lives in the ppj ring (alloc->MMs->evac within one
                # chunk, never across blocks); the psh "shared" tag belongs
                # to zp alone, whose lifetime spans two pipeline blocks
                pf = ppj.tile([128, 512], f32, tag="pp", name="pf")
                for cp in range(8):
                    nc.tensor.matmul(
                        pf[:], wo_sb[:, 1024 * cp + 128 * cf:1024 * cp + 128 * (cf + 1)],
                        out2t[:, 2048 * cp + 512 * fg:2048 * cp + 512 * (fg + 1)],
                        start=(cp == 0), stop=(cp == 7))
                osb = work.tile([128, 512], bf16, tag="osb")
                nc.vector.tensor_scalar_add(osb[:], pf[:], boT[:, cf:cf + 1])
                nc.sync.dma_start(out_d[128 * cf:128 * (cf + 1), 512 * fg:512 * (fg + 1)], osb[:])

        st = {}

        def _stage_scores(sub, qt_sg, kt_sg):
            sl = sub % 4
            # ---- QSTK / KBLK via 4 strided DVE copies each ----
            qstk = work.tile([128, 1024], bf16, tag="qstk", name="qstk")
            kblk = kblks[sub % 2]
            for p in range(2):
                for a in range(2):
                    src_q = qt_sg[:][64 * a:64 * (a + 1), :].rearrange(
                        "P (c sl G cp t) -> P sl t G cp c",
                        c=8, sl=4, G=8, cp=8, t=2)[:, sl, p]
                    dst_q = qstk[:][64 * p:64 * (p + 1), :].rearrange(
                        "P (G cp c t) -> P t G cp c", G=8, cp=8, c=8, t=2)[:, a]
                    nc.vector.tensor_copy(dst_q, src_q)
                    src_k = kt_sg[:][64 * a:64 * (a + 1), :].rearrange(
                        "P (c sl G cp t) -> P sl t G cp c",
                        c=8, sl=4, G=8, cp=8, t=2)[:, sl, p]
                    dst_k = kblk[:][64 * p:64 * (p + 1), :].rearrange(
                        "P (G cp two c t) -> P two t G cp c",
                        G=8, cp=8, two=2, c=8, t=2)[:, p, a]
                    nc.vector.tensor_copy(dst_k, src_k)
            # ---- scores^T: 16 pair-group matmuls; valid rows of each
            # 16-col strip are 32u:+32 (cross-pair garbage, never read) ----
            scot = psc.tile([128, 1024], f32, tag="sco", name="scot", bufs=1)
            for q in range(16):
                nc.tensor.matmul(
                    scot[:, 64 * q:64 * (q + 1)],
                    kblk[:, 128 * q:128 * (q + 1)],
                    qstk[:, 64 * q:64 * (q + 1)],
                    start=True, stop=True)
            if "nosm" in _ab:
                return
            # ---- exp: one ACT op per u-band over all 16 q-groups ----
            e_sb = sfx.tile([128, 256], bf16, tag="esb", name="esb")
            for u in range(4):
                nc.scalar.activation(
                    e_sb[:][32 * u:32 * (u + 1), :],
                    scot[:][32 * u:32 * (u + 1), :].rearrange(
                        "P (q r) -> P q r", q=16)[:, :, 16 * u:16 * (u + 1)],
                    func=AF.Exp)
            st[sub] = {"e": e_sb}

        def _stage_z(sub):
            if sub not in st:
                return
            s = st[sub]
            zp = psh.tile([128, 512], f32, tag="shared", name="zp")
            nc.tensor.matmul(zp[0:32, 0:256], mask_sb[:], s["e"][:],
                             start=True, stop=True)
            zr = zrs[sub % 2]
            with nc.allow_low_precision(reason="1/Z in bf16: 0.4% on softmax scale"):
                nc.vector.reciprocal(zr[0:8, :], zp[0:8, 0:256])
            s["zp"], s["zr"] = zp, zr

        def _stage_bcast(sub):
            if sub not in st:
                return
            s = st[sub]
            nc.tensor.matmul(s["zp"][:, 256:512], maskT_sb[:], s["zr"][:],
                             start=True, stop=True)
            # ---- A2BIG [(u,p,j), 64q+16u+i block-diag in u]: fused
            # normalize + diagonal placement, 4 DVE muls ----
            a2big = a2bigs[sub % 2]
            for u in range(4):
                nc.vector.tensor_mul(
                    a2big[:][32 * u:32 * (u + 1), :].rearrange(
                        "P (q four i) -> P q four i", four=4, i=16)[:, :, u, :],
                    s["e"][:][32 * u:32 * (u + 1), :].rearrange("P (q i) -> P q i", i=16),
                    s["zp"][:][32 * u:32 * (u + 1), 256:512].rearrange("P (q i) -> P q i", i=16))

        def _stage_av(sub):
            if "noav" in _ab or sub not in st:
                return
            st.pop(sub)
            vblk4, a2big = vblk4s[sub % 8], a2bigs[sub % 2]
            # 16 pair-group matmuls: stationary [128,128] = V of 4 pairs
            # (block-diag in p), moving [128,64] = A2BIG slice (block-diag
            # in u) -> avt[(p,d), 64q_loc + 16u + i]
            for h2 in range(2):
                avt = pav.tile([128, 512], f32, tag="avp", name="avt")
                for qq in range(8):
                    q = 8 * h2 + qq
                    nc.tensor.matmul(
                        avt[:, 64 * qq:64 * (qq + 1)],
                        vblk4[:, 128 * q:128 * (q + 1)],
                        a2big[:, 64 * q:64 * (q + 1)],
                        start=True, stop=True)
                # evac half to out2T (one strided ACT op per half)
                # avt col = 128qh + 64ql + 16u + i (q = 8h2 + 2qh + ql);
                # out2t col = 8192ql + 2048u + 128sub + 64h2 + 16qh + i
                #           = 2048c' + 128sub + 16G + i  (c' = 4ql+u, G = 4h2+qh)
                nc.scalar.activation(
                    out2t[:].rearrange("P (ql u sb h2 qh i) -> P sb h2 ql u qh i",
                                       ql=2, u=4, sb=16, h2=2, qh=4)[:, sub, h2],
                    avt[:].rearrange("P (qh ql u i) -> P ql u qh i", qh=4, ql=2, u=4),
                    func=AF.Copy)

        if "onlyweights" in _ab:
            return nc
        for sg in range(4):
            # ---- load xT group in 4 ce-chunks: [128, (ce, s 512)] ----
            xt_sb = work.tile([128, 4096], bf16, tag="xt")
            for cc in range(4):
                nc.sync.dma_start(
                    xt_sb[:].rearrange("p (ce s) -> p ce s", ce=8)[:, 2 * cc:2 * (cc + 1), :],
                    xt_d[:].rearrange("(ce p) s -> p ce s", ce=8)[:, 2 * cc:2 * (cc + 1),
                                                                 512 * sg:512 * (sg + 1)],
                )
            # ---- QT / KT projections (transposed: stationary = weights) ----
            qt_sg = work.tile([128, 4096], bf16, tag="qt", bufs=1)
            kt_sg = work.tile([128, 4096], bf16, tag="kt", bufs=1)
            for w_sb, bT, dst in ((wq_sb, bqT, qt_sg), (wk_sb, bkT, kt_sg)):
                for cf in range(8):
                    pp = ppj.tile([128, 512], f32, tag="pp", name="pp")
                    for ce in range(8):
                        nc.tensor.matmul(
                            pp[:], w_sb[:, 1024 * ce + 128 * cf:1024 * ce + 128 * (cf + 1)],
                            xt_sb[:, 512 * ce:512 * (ce + 1)],
                            start=(ce == 0), stop=(ce == 7))
                    nc.vector.tensor_scalar_add(
                        dst[:, 512 * cf:512 * (cf + 1)], pp[:], bT[:, cf:cf + 1])
            # ---- V projection (natural: stationary = xT chunk), DRAM
            # bounce, then VBLK4 [(u,p,j), 128q+64p'+d block-diag in p] via 8
            # gather DMAs per subtile, issued during the V phase so they
            # spread across the sg (SBUF DMA APs need partition step 1, hence
            # the bounce; gpsimd/SWDGE DMA breaks walrus codegen): subtile
            # row = 8q+2u+p ----
            for sl in range(4):
                sub = 4 * sg + sl
                vn = vnp.tile([128, 1024], bf16, tag="vn")
                for h in range(2):
                    pv = ppj.tile([128, 512], f32, tag="pp", name="pv")
                    for ce in range(8):
                        nc.tensor.matmul(
                            pv[:],
                            xt_sb[:, 512 * ce + 128 * sl:512 * ce + 128 * (sl + 1)],
                            wv_sb[:, 1024 * ce + 512 * h:1024 * ce + 512 * (h + 1)],
                            start=(ce == 0), stop=(ce == 7))
                    nc.vector.tensor_add(vn[:, 512 * h:512 * (h + 1)], pv[:],
                                         bvR[:, 512 * h:512 * (h + 1)])
                if "nogather" in _ab:
                    continue
                vdr = drp.tile([128, 1024], bf16, tag="vdr")
                nc.sync.dma_start(vdr[:], vn[:])
                vblk4 = vblk4s[sub % 8]
                for p in range(2):
                    for u in range(4):
                        nc.sync.dma_start(
                            vblk4[:][32 * u + 16 * p:32 * u + 16 * p + 16, :]
                            .rearrange("P (q two d) -> P q two d", two=2, d=64)[:, :, p, :],
                            vdr[:].rearrange("(q r) (j d) -> r j q d", r=8, j=16)[2 * u + p],
                        )

            # ---- 3-deep skewed pipeline: scores(n) | Z(n-1) | bcast(n-2) |
            # AV(n-3): every PE instruction's inputs are ready ~a full
            # subtile before it issues, so the exp->Z->recip->bcast->mul
            # chain never stalls the in-order PE queue ----
            for sl in range(4):
                sub = 4 * sg + sl
                if "noscores" not in _ab:
                    _stage_scores(sub, qt_sg, kt_sg)
                if sub >= 1:
                    _stage_z(sub - 1)
                if sub >= 2:
                    _stage_bcast(sub - 2)
                if sub >= 3:
                    _stage_av(sub - 3)
                    if (sub - 3) % 4 == 3:
                        _emit_final((sub - 3) // 4)
        # ---- pipeline flush ----
        _stage_z(15)
        _stage_bcast(14)
        _stage_av(13)
        _stage_bcast(15)
        _stage_av(14)
        _stage_av(15)
        _emit_final(3)
    return nc


def _host_prep(x, Wq, bq, Wk, bk, Wv, bv, Wo, bo):
    """Returns per-core input maps."""
    xf = np.ascontiguousarray(x.reshape(NB * SB, EB))
    WqT = np.ascontiguousarray((Wq / 8.0).T).astype(ml_dtypes.bfloat16)
    WkT = np.ascontiguousarray(Wk.T).astype(ml_dtypes.bfloat16)
    WvT = np.ascontiguousarray(Wv.T).astype(ml_dtypes.bfloat16)
    WoPT = np.zeros((1024, 1024), np.float32)
    for cp in range(8):
        for a in range(2):
            w = 2 * cp + a
            WoPT[128 * cp + 64 * a:128 * cp + 64 * a + 64, :] = Wo[:, 64 * w:64 * (w + 1)].T
    WoPT = WoPT.astype(ml_dtypes.bfloat16)
    bias = np.zeros((128, 1048), np.float32)
    bias[:, 0:8] = (bq / 8.0).reshape(8, 128).T
    bias[:, 8:16] = bk.reshape(8, 128).T
    bias[:, 16:24] = bo.reshape(8, 128).T
    bias[:, 24:1048] = np.tile(bv[None, :], (128, 1))
    MASK = np.zeros((128, 32), np.float32)
    for u in range(4):
        for p in range(2):
            MASK[32 * u + 16 * p:32 * u + 16 * (p + 1), 2 * u + p] = 1.0
    MASKb = MASK.astype(ml_dtypes.bfloat16)
    MASKTb = np.ascontiguousarray(MASK.T).astype(ml_dtypes.bfloat16)
    in_maps = []
    for core in range(8):
        n, s0 = core // 2, (core % 2) * ROWS
        xs = xf[n * SB + s0:n * SB + s0 + ROWS]
        xT = np.ascontiguousarray(xs.T).astype(ml_dtypes.bfloat16)
        in_maps.append({"xt": xT, "wqt": WqT, "wkt": WkT, "wvt": WvT,
                        "wot": WoPT, "bias": bias, "mask": MASKb, "maskt": MASKTb})
    return in_maps


def _gather_out(core_outs):
    """core_outs: list of 8 per-core 'out' arrays (1024, 2048) -> full (N,S,E).
    out col = 16*G + i -> row i*256 + s0/16 + G."""
    out = np.zeros((NB, SB, EB), np.float32)
    cols = np.arange(ROWS)
    G, i = cols // 16, cols % 16
    for core in range(8):
        n, s0 = core // 2, (core % 2) * ROWS
        fT = np.asarray(core_outs[core])  # (1024, 2048)
        rows = i * 256 + (s0 // 16 + G)
        out[n, rows, :] = fT.T
    return out


def kernel(x, Wq, bq, Wk, bk, Wv, bv, Wo, bo):
    _install_birpatch()
    from concourse.bass_utils import run_bass_kernel_spmd

    if "nc" not in _CACHE:
        _CACHE["nc"] = _build_bass()
    nc = _CACHE["nc"]
    in_maps = _host_prep(np.asarray(x, np.float32), *[np.asarray(a, np.float32)
                         for a in (Wq, bq, Wk, bk, Wv, bv, Wo, bo)])
    res = run_bass_kernel_spmd(nc, in_maps, list(range(8)))
    return _gather_out([res.results[core]["out"] for core in range(8)])


# revision 31
# speedup vs baseline: 1.0141x; 1.0141x over previous
"""Trainium2 Bass kernel for nn_MultiHeadSelfAttention_62646392979761.

Math (per the buggy-einsum reference): per position s, heads attend to heads:
  Q,K,V = x@W{q,k,v}.T + b  (N,S,H,D);  scores[s] = Q[s]K[s]^T/8 (16x16);
  A = softmax_j;  AV[s] = A[s]V[s];  out2 = scrambled flat reshape;
  final = out2@Wo.T + bo.

Sharding: 8 cores x 2048 rows of the flattened (16384, 1024) x. Attention is
position-local; the scramble groups 16 consecutive positions, which never
cross a 2048-row shard. Zero cross-core communication.

Per-core pipeline (positions in 4 groups of 512, each 4 subtiles of 128).
Indices: position s = 128*sub + 16*G + w, pair-half p = w%2, c' = w//2%8,
pair g = 8*G + c' (u = g%4, q = g//4), head i = 2c+a.
  1. QT/KT projections transposed (stationary = weight chunk, moving = xT):
     psum [128 f=(a,d), 512 s] per f-chunk; DVE bias evac -> qt/kt [(a,d),(c,s)].
  2. V projection natural (stationary = xT chunk): vn [128 s, 1024 (j,d)],
     bounced to DRAM for the VBLK4 gather.
  3. QSTK [(p,d), 16g+i] / KBLK [(p,d), 32g+16p+j block-diag] via 4 strided
     DVE copies each from qt/kt. VBLK4 [(u,p,j), 128q+64p+d block-diag in p]
     via 8 gather DMAs (u,p) from the DRAM bounce: one [128,128] stationary
     slab holds the V of 4 pairs (8 positions). Structural zeros memset per
     rep on GPSIMD.
  4. Scores^T: 16 pair-group matmuls (k=128, m=32 col-quadrant-rotated, n=64,
     stationary KBLK slab, moving QSTK slice) -> sco psum [(u,p,j), (q,i)].
  5. Softmax: ACT exp -> E bf16; Z = mask-matmul (sums j over partitions,
     padded to m=32); DVE reciprocal; Zb = maskT-matmul broadcast; A2BIG
     [(u,p,j), 64q+16u+i block-diag in u] = E*Zb via 4 DVE muls (fused
     normalize + diagonal placement; structural zeros memset per rep).
  6. AV: 16 matmuls per subtile (k=128, m=128, n=64: stationary VBLK4 slab,
     moving A2BIG slice) -> avt psum [(p,d), 64q_loc+16u+i] x2 halves.
     (Baseline did 64 n=16 matmuls w/ [32,128] stationaries: LDWEIGHTS-bound.)
  7. One strided ACT copy per avt half -> out2T [128 (p,d), (c',sub,G,i)].
  Stages run as a 3-deep skewed pipeline: scores(n) | Z(n-1) | bcast(n-2) |
  AV(n-3), so the exp->Z->recip->bcast->mul chain of subtile n resolves a
  full subtile before any PE instruction that needs it issues (the PE queue
  is in-order; un-skewed, Z/bcast head-of-line-block the ready AV matmuls).
  Ring depths follow the skew: vblk4 x8 (a whole sg's gathers are emitted in
  the sg head, 4 ahead of their AV), a2big/zr/kblk x2, esb x4. zr is a
  memset-once 32-row const ring (8-row matmul operands FAULT real HW). zp
  keeps its own psum tag; pf shares the ppj ring instead (a same-tag ring is
  only safe when no allocation interleaves a live tile's alloc->last-read
  window).
  8. Final projection: stationary WoPT chunks, moving out2T -> psum
     [128 f, 512 s'], + bo (DVE) -> out_d (1024, 2048) f32.
Weights/bias/masks load on the Activation HWDGE queue (overlaps the rep
boundary: their WAR hazards clear mid-rep, so the reload for rep k+1 runs
under rep k's tail compute); xt loads split into 4 ce-chunks on the SP queue
so rep k+1's first QK matmuls start ~1us after rep k's stores issue.
Host: transposes x/weights, permutes Wo rows, post-scatters out columns
(col = 16G + i -> row i*256 + s0/16 + G).
Note: row-quadrant tile_position (32u, 0) on AV matmuls compiles but faults
real HW (NRT_EXEC_UNIT_UNRECOVERABLE) — everything here stays at (0,0).
"""

import math
import os
import numpy as np
import ml_dtypes

os.environ.setdefault("NEURON_RT_RESET_CORES", "1")

ROWS = 2048
NB, SB, EB, HB, DB = 4, 4096, 1024, 16, 64

_CACHE = {}


def _split_waits_json(bir_bytes):
    """This env's walrus accepts only ONE embedded sync-wait per TPB
    instruction (NEURON_ISA_TPB_EVENTS has a single wait slot) but Tile emits
    several. Split excess on_wait entries onto standalone EventSemaphore
    instructions inserted just before, on the same engine — semantically
    identical on in-order engine queues."""
    import json
    d = json.loads(bir_bytes)
    for fn in d.get('functions', []):
        for bb in (fn.get('basic_blocks') or fn.get('blocks') or []):
            out = []
            for inst in bb.get('instructions', []):
                si = inst.get('sync_info')
                w = (si or {}).get('on_wait') or []
                if len(w) > 1:
                    for k, extra in enumerate(w[:-1]):
                        out.append({
                            'debug': inst.get('debug', 0),
                            'engine': inst['engine'],
                            'ins': [], 'outs': [],
                            'name': f"{inst['name']}-sw{k}",
                            'opcode': 'EventSemaphore',
                            'sync_info': {'on_wait': [extra], 'on_update': []},
                        })
                    si['on_wait'] = [w[-1]]
                out.append(inst)
            bb['instructions'] = out
    return json.dumps(d).encode()


def _install_birpatch():
    import concourse.bass_utils as bu
    import concourse.bass2jax as b2j
    if getattr(bu.compile_bir_kernel, '_waitsplit', False):
        return
    orig = bu.compile_bir_kernel

    def patched(bir_json, tmpdir, neff_name="file.neff"):
        return orig(_split_waits_json(bir_json), tmpdir, neff_name)

    patched._waitsplit = True
    bu.compile_bir_kernel = patched
    b2j.compile_bir_kernel = patched


def _build_bass(reps=1):
    import concourse.bass as bass
    import concourse.tile as tile
    from concourse import mybir
    # timing ablations (never set in the graded path): noav, nosm, noscores,
    # nogather strip stages to attribute HW time per stage
    _ab = set(os.environ.get("KABLATE", "").split(","))

    bf16 = mybir.dt.bfloat16
    f32 = mybir.dt.float32
    AF = mybir.ActivationFunctionType

    nc = bass.Bass(trn_type="TRN2")
    xt_d = nc.declare_dram_parameter("xt", [1024, ROWS], bf16, isOutput=False)
    wqT_d = nc.declare_dram_parameter("wqt", [1024, 1024], bf16, isOutput=False)
    wkT_d = nc.declare_dram_parameter("wkt", [1024, 1024], bf16, isOutput=False)
    wvT_d = nc.declare_dram_parameter("wvt", [1024, 1024], bf16, isOutput=False)
    woT_d = nc.declare_dram_parameter("wot", [1024, 1024], bf16, isOutput=False)
    bias_d = nc.declare_dram_parameter("bias", [128, 1048], f32, isOutput=False)
    mask_d = nc.declare_dram_parameter("mask", [128, 32], bf16, isOutput=False)
    maskT_d = nc.declare_dram_parameter("maskt", [32, 128], bf16, isOutput=False)
    out_d = nc.declare_dram_parameter("out", [1024, ROWS], bf16, isOutput=True)

    from contextlib import ExitStack
    with ExitStack() as ctx:
        tc = ctx.enter_context(tile.TileContext(nc))
        const = ctx.enter_context(tc.tile_pool(name="const", bufs=1))
        work = ctx.enter_context(tc.tile_pool(name="work", bufs=2))
        sfx = ctx.enter_context(tc.tile_pool(name="sfx", bufs=4))
        vnp = ctx.enter_context(tc.tile_pool(name="vnp", bufs=3))
        ppj = ctx.enter_context(tc.tile_pool(name="ppj", bufs=2, space="PSUM"))
        psc = ctx.enter_context(tc.tile_pool(name="psc", bufs=2, space="PSUM"))
        pav = ctx.enter_context(tc.tile_pool(name="pav", bufs=2, space="PSUM"))
        psh = ctx.enter_context(tc.tile_pool(name="psh", bufs=2, space="PSUM"))
        drp = ctx.enter_context(tc.tile_pool(name="drp", bufs=4, space="DRAM"))

        if reps != 1:
            ctx.enter_context(tc.For_i(0, reps))

        # ---- persistent tensors (reloaded every rep for honest timing) ----
        wq_sb = const.tile([128, 8192], bf16, tag="wq")
        wk_sb = const.tile([128, 8192], bf16, tag="wk")
        wv_sb = const.tile([128, 8192], bf16, tag="wv")
        wo_sb = const.tile([128, 8192], bf16, tag="wo")
        bias_sb = const.tile([128, 24], f32, tag="bias")
        biasv_sb = const.tile([128, 1024], f32, tag="biasv")
        mask_sb = const.tile([128, 32], bf16, tag="mask")
        maskT_sb = const.tile([32, 128], bf16, tag="maskt")
        out2t = const.tile([128, 16384], bf16, tag="o2t", name="o2t")
        kblks = [const.tile([128, 2048], bf16, tag=f"kblk{b}", name=f"kblk{b}")
                 for b in range(2)]
        vblk4s = [const.tile([128, 2048], bf16, tag=f"vblk4{b}", name=f"vblk4{b}")
                  for b in range(8)]
        a2bigs = [const.tile([128, 1024], bf16, tag=f"a2big{b}", name=f"a2big{b}")
                  for b in range(2)]
        zrs = [const.tile([32, 256], bf16, tag=f"zr{b}", name=f"zr{b}")
               for b in range(2)]
        for b in range(2):
            nc.gpsimd.memset(zrs[b][:], 0.0)
        for b in range(2):
            nc.gpsimd.memset(kblks[b][:], 0.0)
            nc.gpsimd.memset(a2bigs[b][:], 0.0)
        for b in range(8):
            nc.gpsimd.memset(vblk4s[b][:], 0.0)

        # weights + bias/masks on the ACT HWDGE queue: frees the SP queue for
        # activations and lets the rep-(k+1) reload overlap rep k's tail.
        # Queue order matters in the For_i steady state (in-order queue, each
        # DMA also waits its WAR hazard): tiny qkbo-bias first (WAR clears at
        # rep end but loads in ~30ns), then wq/wk (WAR ~75% of prior rep),
        # v-bias, wv, wo, masks.
        nc.scalar.dma_start(bias_sb[:], bias_d[:, 0:24])
        # weight layout: sb[p, 1024*ce + 128*cf + m] = W_T[128*ce + p, 128*cf + m]
        for sb, d in ((wq_sb, wqT_d), (wk_sb, wkT_d)):
            nc.scalar.dma_start(
                sb[:].rearrange("p (ce f) -> p ce f", ce=8),
                d[:].rearrange("(ce p) f -> p ce f", ce=8),
            )
        nc.scalar.dma_start(biasv_sb[:], bias_d[:, 24:1048])
        for sb, d in ((wv_sb, wvT_d), (wo_sb, woT_d)):
            nc.scalar.dma_start(
                sb[:].rearrange("p (ce f) -> p ce f", ce=8),
                d[:].rearrange("(ce p) f -> p ce f", ce=8),
            )
        nc.scalar.dma_start(mask_sb[:], mask_d[:])
        nc.scalar.dma_start(maskT_sb[:], maskT_d[:])
        bqT = bias_sb[:, 0:8]
        bkT = bias_sb[:, 8:16]
        boT = bias_sb[:, 16:24]
        bvR = biasv_sb[:, 0:1024]

        def _emit_final(fg):
            if "nofinal" in _ab:
                return
            for cf in range(8):
                # pf lives in the ppj ring (alloc->MMs->evac within one
                # chunk, never across blocks); the psh "shared" tag belongs
                # to zp alone, whose lifetime spans two pipeline blocks
                pf = ppj.tile([128, 512], f32, tag="pp", name="pf")
                for cp in range(8):
                    nc.tensor.matmul(
                        pf[:], wo_sb[:, 1024 * cp + 128 * cf:1024 * cp + 128 * (cf + 1)],
                        out2t[:, 2048 * cp + 512 * fg:2048 * cp + 512 * (fg + 1)],
                        start=(cp == 0), stop=(cp == 7))
                osb = work.tile([128, 512], bf16, tag="osb")
                nc.vector.tensor_scalar_add(osb[:], pf[:], boT[:, cf:cf + 1])
                nc.sync.dma_start(out_d[128 * cf:128 * (cf + 1), 512 * fg:512 * (fg + 1)], osb[:])

        st = {}

        def _stage_scores(sub, qt_sg, kt_sg):
            sl = sub % 4
            # ---- QSTK / KBLK via 4 strided DVE copies each ----
            qstk = work.tile([128, 1024], bf16, tag="qstk", name="qstk")
            kblk = kblks[sub % 2]
            for p in range(2):
                for a in range(2):
                    src_q = qt_sg[:][64 * a:64 * (a + 1), :].rearrange(
                        "P (c sl G cp t) -> P sl t G cp c",
                        c=8, sl=4, G=8, cp=8, t=2)[:, sl, p]
                    dst_q = qstk[:][64 * p:64 * (p + 1), :].rearrange(
                        "P (G cp c t) -> P t G cp c", G=8, cp=8, c=8, t=2)[:, a]
                    nc.vector.tensor_copy(dst_q, src_q)
                    src_k = kt_sg[:][64 * a:64 * (a + 1), :].rearrange(
                        "P (c sl G cp t) -> P sl t G cp c",
                        c=8, sl=4, G=8, cp=8, t=2)[:, sl, p]
                    dst_k = kblk[:][64 * p:64 * (p + 1), :].rearrange(
                        "P (G cp two c t) -> P two t G cp c",
                        G=8, cp=8, two=2, c=8, t=2)[:, p, a]
                    nc.vector.tensor_copy(dst_k, src_k)
            # ---- scores^T: 16 pair-group matmuls; valid rows of each
            # 16-col strip are 32u:+32 (cross-pair garbage, never read) ----
            sco_h = []
            for h in range(2):
                scot = psc.tile([128, 512], f32, tag="sco", name="scot")
                sco_h.append(scot)
                for qq in range(8):
                    q = 8 * h + qq
                    nc.tensor.matmul(
                        scot[:, 64 * qq:64 * (qq + 1)],
                        kblk[:, 128 * q:128 * (q + 1)],
                        qstk[:, 64 * q:64 * (q + 1)],
                        start=True, stop=True)
            if "nosm" in _ab:
                return
            # ---- exp ----
            e_sb = sfx.tile([128, 256], bf16, tag="esb", name="esb")
            for h in range(2):
                for u in range(4):
                    nc.scalar.activation(
                        e_sb[:][32 * u:32 * (u + 1), 128 * h:128 * (h + 1)],
                        sco_h[h][:][32 * u:32 * (u + 1), :].rearrange(
                            "P (qq r) -> P qq r", qq=8)[:, :, 16 * u:16 * (u + 1)],
                        func=AF.Exp)
            st[sub] = {"e": e_sb}

        def _stage_z(sub):
            if sub not in st:
                return
            s = st[sub]
            zp = psh.tile([128, 512], f32, tag="shared", name="zp")
            nc.tensor.matmul(zp[0:32, 0:256], mask_sb[:], s["e"][:],
                             start=True, stop=True)
            zr = zrs[sub % 2]
            with nc.allow_low_precision(reason="1/Z in bf16: 0.4% on softmax scale"):
                nc.vector.reciprocal(zr[0:8, :], zp[0:8, 0:256])
            s["zp"], s["zr"] = zp, zr

        def _stage_bcast(sub):
            if sub not in st:
                return
            s = st[sub]
            nc.tensor.matmul(s["zp"][:, 256:512], maskT_sb[:], s["zr"][:],
                             start=True, stop=True)
            # ---- A2BIG [(u,p,j), 64q+16u+i block-diag in u]: fused
            # normalize + diagonal placement, 4 DVE muls ----
            a2big = a2bigs[sub % 2]
            for u in range(4):
                nc.vector.tensor_mul(
                    a2big[:][32 * u:32 * (u + 1), :].rearrange(
                        "P (q four i) -> P q four i", four=4, i=16)[:, :, u, :],
                    s["e"][:][32 * u:32 * (u + 1), :].rearrange("P (q i) -> P q i", i=16),
                    s["zp"][:][32 * u:32 * (u + 1), 256:512].rearrange("P (q i) -> P q i", i=16))

        def _stage_av(sub):
            if "noav" in _ab or sub not in st:
                return
            st.pop(sub)
            vblk4, a2big = vblk4s[sub % 8], a2bigs[sub % 2]
            # 16 pair-group matmuls: stationary [128,128] = V of 4 pairs
            # (block-diag in p), moving [128,64] = A2BIG slice (block-diag
            # in u) -> avt[(p,d), 64q_loc + 16u + i]
            for h2 in range(2):
                avt = pav.tile([128, 512], f32, tag="avp", name="avt")
                for qq in range(8):
                    q = 8 * h2 + qq
                    nc.tensor.matmul(
                        avt[:, 64 * qq:64 * (qq + 1)],
                        vblk4[:, 128 * q:128 * (q + 1)],
                        a2big[:, 64 * q:64 * (q + 1)],
                        start=True, stop=True)
                # evac half to out2T (one strided ACT op per half)
                # avt col = 128qh + 64ql + 16u + i (q = 8h2 + 2qh + ql);
                # out2t col = 8192ql + 2048u + 128sub + 64h2 + 16qh + i
                #           = 2048c' + 128sub + 16G + i  (c' = 4ql+u, G = 4h2+qh)
                nc.scalar.activation(
                    out2t[:].rearrange("P (ql u sb h2 qh i) -> P sb h2 ql u qh i",
                                       ql=2, u=4, sb=16, h2=2, qh=4)[:, sub, h2],
                    avt[:].rearrange("P (qh ql u i) -> P ql u qh i", qh=4, ql=2, u=4),
                    func=AF.Copy)

        if "onlyweights" in _ab:
            return nc
        for sg in range(4):
            # ---- load xT group in 4 ce-chunks: [128, (ce, s 512)] ----
            xt_sb = work.tile([128, 4096], bf16, tag="xt")
            for cc in range(4):
                nc.sync.dma_start(
                    xt_sb[:].rearrange("p (ce s) -> p ce s", ce=8)[:, 2 * cc:2 * (cc + 1), :],
                    xt_d[:].rearrange("(ce p) s -> p ce s", ce=8)[:, 2 * cc:2 * (cc + 1),
                                                                 512 * sg:512 * (sg + 1)],
                )
            # ---- QT / KT projections (transposed: stationary = weights) ----
            qt_sg = work.tile([128, 4096], bf16, tag="qt", bufs=1)
            kt_sg = work.tile([128, 4096], bf16, tag="kt", bufs=1)
            for w_sb, bT, dst in ((wq_sb, bqT, qt_sg), (wk_sb, bkT, kt_sg)):
                for cf in range(8):
                    pp = ppj.tile([128, 512], f32, tag="pp", name="pp")
                    for ce in range(8):
                        nc.tensor.matmul(
                            pp[:], w_sb[:, 1024 * ce + 128 * cf:1024 * ce + 128 * (cf + 1)],
                            xt_sb[:, 512 * ce:512 * (ce + 1)],
                            start=(ce == 0), stop=(ce == 7))
                    nc.scalar.activation(
                        dst[:, 512 * cf:512 * (cf + 1)], pp[:],
                        func=AF.Identity, bias=bT[:, cf:cf + 1])
            # ---- V projection (natural: stationary = xT chunk), DRAM
            # bounce, then VBLK4 [(u,p,j), 128q+64p'+d block-diag in p] via 8
            # gather DMAs per subtile, issued during the V phase so they
            # spread across the sg (SBUF DMA APs need partition step 1, hence
            # the bounce; gpsimd/SWDGE DMA breaks walrus codegen): subtile
            # row = 8q+2u+p ----

            # ---- 3-deep skewed pipeline: scores(n) | Z(n-1) | bcast(n-2) |
            # AV(n-3): every PE instruction's inputs are ready ~a full
            # subtile before it issues, so the exp->Z->recip->bcast->mul
            # chain never stalls the in-order PE queue ----
            for sl in range(4):
                sub = 4 * sg + sl
                vn = vnp.tile([128, 1024], bf16, tag="vn")
                for h in range(2):
                    pv = ppj.tile([128, 512], f32, tag="pp", name="pv")
                    for ce in range(8):
                        nc.tensor.matmul(
                            pv[:],
                            xt_sb[:, 512 * ce + 128 * sl:512 * ce + 128 * (sl + 1)],
                            wv_sb[:, 1024 * ce + 512 * h:1024 * ce + 512 * (h + 1)],
                            start=(ce == 0), stop=(ce == 7))
                    nc.vector.tensor_add(vn[:, 512 * h:512 * (h + 1)], pv[:],
                                         bvR[:, 512 * h:512 * (h + 1)])
                if "nogather" not in _ab:
                    vdr = drp.tile([128, 1024], bf16, tag="vdr")
                    nc.sync.dma_start(vdr[:], vn[:])
                    vblk4 = vblk4s[sub % 8]
                    for p in range(2):
                        for u in range(4):
                            nc.sync.dma_start(
                                vblk4[:][32 * u + 16 * p:32 * u + 16 * p + 16, :]
                                .rearrange("P (q two d) -> P q two d", two=2, d=64)[:, :, p, :],
                                vdr[:].rearrange("(q r) (j d) -> r j q d", r=8, j=16)[2 * u + p],
                            )
                if "noscores" not in _ab:
                    _stage_scores(sub, qt_sg, kt_sg)
                if sub >= 1:
                    _stage_z(sub - 1)
                if sub >= 2:
                    _stage_bcast(sub - 2)
                if sub >= 3:
                    _stage_av(sub - 3)
                    if (sub - 3) % 4 == 3:
                        _emit_final((sub - 3) // 4)
        # ---- pipeline flush ----
        _stage_z(15)
        _stage_bcast(14)
        _stage_av(13)
        _stage_bcast(15)
        _stage_av(14)
        _stage_av(15)
        _emit_final(3)
    return nc


def _host_prep(x, Wq, bq, Wk, bk, Wv, bv, Wo, bo):
    """Returns per-core input maps."""
    xf = np.ascontiguousarray(x.reshape(NB * SB, EB))
    WqT = np.ascontiguousarray((Wq / 8.0).T).astype(ml_dtypes.bfloat16)
    WkT = np.ascontiguousarray(Wk.T).astype(ml_dtypes.bfloat16)
    WvT = np.ascontiguousarray(Wv.T).astype(ml_dtypes.bfloat16)
    WoPT = np.zeros((1024, 1024), np.float32)
    for cp in range(8):
        for a in range(2):
            w = 2 * cp + a
            WoPT[128 * cp + 64 * a:128 * cp + 64 * a + 64, :] = Wo[:, 64 * w:64 * (w + 1)].T
    WoPT = WoPT.astype(ml_dtypes.bfloat16)
    bias = np.zeros((128, 1048), np.float32)
    bias[:, 0:8] = (bq / 8.0).reshape(8, 128).T
    bias[:, 8:16] = bk.reshape(8, 128).T
    bias[:, 16:24] = bo.reshape(8, 128).T
    bias[:, 24:1048] = np.tile(bv[None, :], (128, 1))
    MASK = np.zeros((128, 32), np.float32)
    for u in range(4):
        for p in range(2):
            MASK[32 * u + 16 * p:32 * u + 16 * (p + 1), 2 * u + p] = 1.0
    MASKb = MASK.astype(ml_dtypes.bfloat16)
    MASKTb = np.ascontiguousarray(MASK.T).astype(ml_dtypes.bfloat16)
    in_maps = []
    for core in range(8):
        n, s0 = core // 2, (core % 2) * ROWS
        xs = xf[n * SB + s0:n * SB + s0 + ROWS]
        xT = np.ascontiguousarray(xs.T).astype(ml_dtypes.bfloat16)
        in_maps.append({"xt": xT, "wqt": WqT, "wkt": WkT, "wvt": WvT,
                        "wot": WoPT, "bias": bias, "mask": MASKb, "maskt": MASKTb})
    return in_maps


def _gather_out(core_outs):
    """core_outs: list of 8 per-core 'out' arrays (1024, 2048) -> full (N,S,E).
    out col = 16*G + i -> row i*256 + s0/16 + G."""
    out = np.zeros((NB, SB, EB), np.float32)
    cols = np.arange(ROWS)
    G, i = cols // 16, cols % 16
    for core in range(8):
        n, s0 = core // 2, (core % 2) * ROWS
        fT = np.asarray(core_outs[core])  # (1024, 2048)
        rows = i * 256 + (s0 // 16 + G)
        out[n, rows, :] = fT.T
    return out


def kernel(x, Wq, bq, Wk, bk, Wv, bv, Wo, bo):
    _install_birpatch()
    from concourse.bass_utils import run_bass_kernel_spmd

    if "nc" not in _CACHE:
        _CACHE["nc"] = _build_bass()
    nc = _CACHE["nc"]
    in_maps = _host_prep(np.asarray(x, np.float32), *[np.asarray(a, np.float32)
                         for a in (Wq, bq, Wk, bk, Wv, bv, Wo, bo)])
    res = run_bass_kernel_spmd(nc, in_maps, list(range(8)))
    return _gather_out([res.results[core]["out"] for core in range(8)])
